# revision 1
# baseline (speedup 1.0000x reference)
"""Trainium2 Bass kernel for nn_Criterion_74448963109285 (segment_reduce criterion).

Strategy (pure data parallel, 2 images per core on 8 cores):
  Per image the loss is  intra + inter + ce  where every term reduces to a
  handful of tiny quantities:
    - segment sums over pixels per label l in {1,2}:
        t_l[e] = sum_{p: lab=l} emb[e,p]          (raw)
        s_l[e] = sum_{p: lab=l} emb[e,p]/||emb_p|| (normalized)
        c_l    = count of pixels with label l
    - ce partials: sum_p logsumexp(pred[:,p]) and sum_p pred[lab_p, p]
  The device computes only these reductions; the final scalar math runs on
  host in float64.

  Layouts: emb (32, 512*512) per image is viewed as 4 pixel groups x 32
  channels = 128 partitions x 65536 columns.  A DVE 32x32 stream-transpose
  turns each tile into pixel-major layout (partition = pixel-in-32-block),
  so the TensorEngine can contract over pixels: per 8 x 32-pixel column
  blocks, matmul( lhsT = w[:, c0:c0+8, 4 weights], rhs = Xt[:, c0:c0+8,
  32 ch + ones] ) accumulates all segment sums and counts into one
  (32, 264) PSUM tile (valid entries on the 8 diagonal blocks).
  Weights w = {oh1, oh2, oh1*inv, oh2*inv} are built from labels (same
  transposed layout) and inv = 1/max(||pixel||, 1e-8) from a squared-tile
  segmented reduce.
"""

import numpy as np

import concourse.bass as bass
import concourse.tile as tile
from concourse import mybir
from concourse.bass_utils import run_bass_kernel_spmd

F32 = mybir.dt.float32
BF16 = mybir.dt.bfloat16
I32 = mybir.dt.int32
ALU = mybir.AluOpType
ACTF = mybir.ActivationFunctionType

B, E, H, W, L = 16, 32, 512, 512, 3
P = H * W                  # 262144 pixels per image
NCORES = 8
BLOC = B // NCORES         # 2 images per core
G = 4                      # pixel groups packed into partitions (4*32ch=128)
PG = P // G                # 65536 pixels per group
NT = 16                    # tiles per image
FCOLS = PG // NT           # 2048 pixel columns per tile (per group)
CB = FCOLS // 32           # 64 c-blocks (32 px each) per tile
CIMG = PG // 32            # 2048 c-blocks per image
TGRP = 2                   # tiles per small-op batch group
CGRP = 8                   # c-blocks per matmul (N = 8*33 = 264)
MM_M = 4 * CGRP            # 32 output partitions
MM_N = 33 * CGRP           # 264 output cols
RES_COLS = 272
PCOLS = P // 128            # 2048 label/pred columns per image
XT_STRIDE = 34             # 32 ch + ones col + pad (keeps runs 4B aligned)


def _split_oversized_waits(nc, max_waits=1):
    """This walrus build accepts only one sync wait per instruction; move
    extra waits onto single-wait NOPs preceding the instruction."""
    for fn in nc.m.functions:
        for blk in fn.blocks:
            new_list = []
            for ins in blk.instructions:
                si = getattr(ins, "sync_info", None)
                if si is not None and si.on_wait and len(si.on_wait) > max_waits:
                    waits = list(si.on_wait)
                    chunks = [
                        waits[i : i + max_waits]
                        for i in range(0, len(waits), max_waits)
                    ]
                    for j, ch in enumerate(chunks[:-1]):
                        new_list.append(
                            mybir.InstNoOp(
                                name=f"{ins.name}-wsplit{j}",
                                engine=ins.engine,
                                sync_info=mybir.SyncInfo(on_wait=ch, on_update=[]),
                                bass_nofuse=True,
                            )
                        )
                    si.on_wait = chunks[-1]
                new_list.append(ins)
            blk.instructions[:] = new_list


def build_nc():
    nc = bass.Bass()
    emb_h = nc.declare_dram_parameter("emb", [BLOC, E, P], F32, isOutput=False)
    pred_h = nc.declare_dram_parameter("pred", [BLOC, L, P], F32, isOutput=False)
    lab_h = nc.declare_dram_parameter("lab", [BLOC, P], I32, isOutput=False)
    res_h = nc.declare_dram_parameter("res", [BLOC, 128, RES_COLS], F32, isOutput=True)

    with tile.TileContext(nc) as tc:
        with (
            tc.tile_pool(name="px", bufs=2) as px,           # f32 emb tiles
            tc.tile_pool(name="pxb", bufs=2) as pxb,         # bf16 emb tiles
            tc.tile_pool(name="pxt", bufs=TGRP + 1) as pxt,  # transposed emb
            tc.tile_pool(name="pxt2", bufs=3) as pxt2,       # squared transposed
            tc.tile_pool(name="pgi", bufs=2) as pgi,         # per-group inv chunks
            tc.tile_pool(name="pimg", bufs=1) as pimg,       # per-image label/weight
            tc.tile_pool(name="pce", bufs=1) as pce,         # CE pred tiles
            tc.tile_pool(name="pcet", bufs=6) as pcet,       # CE temporaries
            tc.tile_pool(name="pres", bufs=2) as pres,
            tc.tile_pool(name="ppsum", bufs=2, space="PSUM") as ppsum,
        ):
            for img in range(BLOC):
                # (g, e, n) DRAM view; DMA pairs it with the (128, n) SBUF tile
                emb_r = emb_h[img].rearrange("e (g n) -> e g n", g=G).rearrange(
                    "e g n -> g e n"
                )

                # ---- labels: load, cast, transpose to pixel-major ----
                lab_i = pimg.tile([128, PCOLS], I32, tag="lab_i")
                nc.sync.dma_start(lab_i[:], lab_h[img].rearrange("(q n) -> q n", q=128))
                lab_b = pimg.tile([128, PCOLS], BF16, tag="lab_b")
                nc.vector.tensor_copy(lab_b[:], lab_i[:])
                lab32 = pimg.tile([128, PCOLS], BF16, tag="lab32")
                # col-permuted out AP: pixel g*PG + c*32 + p' lands at [g*32+p', c]
                nc.vector.transpose(
                    lab32[:].rearrange("p (r j) -> p j r", r=32), lab_b[:]
                )

                # ---- weights w[:, c, m]: {oh1, oh2, oh1*inv, oh2*inv} ----
                w = pimg.tile([128, CIMG, 4], BF16, tag="w")
                nc.vector.tensor_scalar(w[:, :, 0], lab32[:], 1.0, None, ALU.is_equal)
                nc.vector.tensor_scalar(w[:, :, 1], lab32[:], 2.0, None, ALU.is_equal)

                nrm2 = pimg.tile([128, CIMG], F32, tag="nrm2")
                acc = ppsum.tile([MM_M, MM_N], F32, tag="acc")

                for tg in range(NT // TGRP):  # 8 groups of 4 tiles
                    xts = []
                    for ti in range(TGRP):
                        t = tg * TGRP + ti
                        x = px.tile([128, FCOLS], F32, tag="x")
                        for g in range(G):
                            nc.sync.dma_start(
                                x[g * E : (g + 1) * E, :],
                                emb_h[
                                    img,
                                    :,
                                    g * PG + t * FCOLS : g * PG + (t + 1) * FCOLS,
                                ],
                            )
                        xb = pxb.tile([128, FCOLS], BF16, tag="xb")
                        nc.scalar.activation(xb[:], x[:], ACTF.Copy)
                        xt = pxt.tile([128, CB, XT_STRIDE], BF16, tag="xt")
                        nc.vector.memset(xt[:, :, 32:33], 1.0)
                        nc.vector.transpose(xt[:, :, 0:32], xb[:])
                        xts.append(xt)
                        xt2 = pxt2.tile([128, CB, 32], BF16, tag="xt2")
                        nc.scalar.activation(xt2[:], xt[:, :, 0:32], ACTF.Square)
                        nc.vector.tensor_reduce(
                            nrm2[:, t * CB : (t + 1) * CB],
                            xt2[:],
                            mybir.AxisListType.X,
                            ALU.add,
                        )

                    # batched small ops over this group's c-range
                    gsl = slice(tg * TGRP * CB, (tg + 1) * TGRP * CB)
                    gn = TGRP * CB  # 256
                    rt = pgi.tile([128, gn], F32, tag="rt")
                    nc.scalar.activation(rt[:], nrm2[:, gsl], ACTF.Sqrt)
                    rtc = pgi.tile([128, gn], F32, tag="rtc")
                    nc.vector.tensor_scalar_max(rtc[:], rt[:], 1e-8)
                    inv = pgi.tile([128, gn], BF16, tag="inv")
                    with nc.allow_low_precision("inv bf16 is plenty for weights"):
                        nc.vector.reciprocal(inv[:], rtc[:])
                    nc.vector.tensor_mul(w[:, gsl, 2], w[:, gsl, 0], inv[:])
                    nc.vector.tensor_mul(w[:, gsl, 3], w[:, gsl, 1], inv[:])

                    for ti in range(TGRP):
                        t = tg * TGRP + ti
                        for mi in range(CB // CGRP):  # 8 matmuls per tile
                            c0 = t * CB + mi * CGRP
                            nc.tensor.matmul(
                                acc[:, :],
                                w[:, c0 : c0 + CGRP, :],
                                xts[ti][:, mi * CGRP : (mi + 1) * CGRP, 0:33],
                                start=(t == 0 and mi == 0),
                                stop=(t == NT - 1 and mi == CB // CGRP - 1),
                            )

                # ---- cross-entropy partials ----
                res = pres.tile([128, RES_COLS], F32, tag="res")
                nc.vector.memset(res[:], 0.0)

                pch = []
                for c in range(L):
                    pc = pce.tile([128, PCOLS], F32, tag=f"p{c}")
                    nc.sync.dma_start(
                        pc[:], pred_h[img, c].rearrange("(q n) -> q n", q=128)
                    )
                    pch.append(pc)
                e0 = pcet.tile([128, PCOLS], BF16, tag="cet")
                nc.scalar.activation(e0[:], pch[0][:], ACTF.Exp)
                e1 = pcet.tile([128, PCOLS], BF16, tag="cet")
                nc.scalar.activation(e1[:], pch[1][:], ACTF.Exp)
                e2 = pcet.tile([128, PCOLS], BF16, tag="cet")
                nc.scalar.activation(e2[:], pch[2][:], ACTF.Exp)
                s01 = pcet.tile([128, PCOLS], BF16, tag="cet")
                nc.vector.tensor_add(s01[:], e0[:], e1[:])
                s012 = pcet.tile([128, PCOLS], BF16, tag="cet")
                nc.vector.tensor_add(s012[:], s01[:], e2[:])
                lntrash = pcet.tile([128, PCOLS], BF16, tag="cet")
                nc.scalar.activation(
                    lntrash[:], s012[:], ACTF.Ln, accum_out=res[:, 264:265]
                )
                # picked = p0 + oh1*(p1-p0) + oh2*(p2-p0); accumulate the parts
                d1 = pcet.tile([128, PCOLS], BF16, tag="cet")
                nc.vector.tensor_sub(d1[:], pch[1][:], pch[0][:])
                d2 = pcet.tile([128, PCOLS], BF16, tag="cet")
                nc.vector.tensor_sub(d2[:], pch[2][:], pch[0][:])
                oc1 = pcet.tile([128, PCOLS], BF16, tag="cet")
                nc.vector.tensor_scalar(oc1[:], lab_b[:], 1.0, None, ALU.is_equal)
                oc2 = pcet.tile([128, PCOLS], BF16, tag="cet")
                nc.vector.tensor_scalar(oc2[:], lab_b[:], 2.0, None, ALU.is_equal)
                nc.vector.tensor_reduce(
                    res[:, 265:266], pch[0][:], mybir.AxisListType.X, ALU.add
                )
                tr1 = pcet.tile([128, PCOLS], BF16, tag="cet")
                nc.vector.scalar_tensor_tensor(
                    tr1[:], d1[:], 1.0, oc1[:], ALU.bypass, ALU.mult,
                    accum_out=res[:, 266:267],
                )
                tr2 = pcet.tile([128, PCOLS], BF16, tag="cet")
                nc.vector.scalar_tensor_tensor(
                    tr2[:], d2[:], 1.0, oc2[:], ALU.bypass, ALU.mult,
                    accum_out=res[:, 267:268],
                )

                nc.vector.tensor_copy(res[0:MM_M, 0:MM_N], acc[:])
                nc.sync.dma_start(res_h[img], res[:])

    _split_oversized_waits(nc)
    return nc


_NC_CACHE = None


def _get_nc():
    global _NC_CACHE
    if _NC_CACHE is None:
        _NC_CACHE = build_nc()
    return _NC_CACHE


def _host_epilogue(res, neighbor):
    """res: (128, RES_COLS) f32 device partials for one image; neighbor (L, 3)."""
    res = res.astype(np.float64)
    A = res[0:MM_M, 0:MM_N]
    M4 = np.zeros((4, 33))
    for cp in range(CGRP):
        M4 += A[cp * 4 : (cp + 1) * 4, cp * 33 : (cp + 1) * 33]
    t1, t2, s1, s2 = M4[0, 0:32], M4[1, 0:32], M4[2, 0:32], M4[3, 0:32]
    c1, c2 = M4[0, 32], M4[1, 32]

    lse_sum = res[:, 264].sum()
    picked_sum = res[:, 265].sum() + res[:, 266].sum() + res[:, 267].sum()
    ce = (lse_sum - picked_sum) / P

    m1, m2 = t1 / c1, t2 / c2
    nm1 = m1 / max(np.linalg.norm(m1), 1e-12)
    nm2 = m2 / max(np.linalg.norm(m2), 1e-12)
    intra = ((1.0 - nm1 @ s1 / c1) + (1.0 - nm2 @ s2 / c2)) / (L - 1)

    nm = np.zeros((L, E))
    nm[1], nm[2] = nm1, nm2
    S = nm @ nm.T
    nb = neighbor.astype(np.int64)
    valid = np.cumprod((nb != 0).astype(np.float64), axis=1)
    rows = np.broadcast_to(np.arange(L)[:, None], nb.shape)
    row_ok = (rows >= 1).astype(np.float64)
    mask = np.zeros((L, L))
    np.maximum.at(mask, (rows.ravel(), nb.ravel()), (valid * row_ok).ravel())
    inter = (S * mask).sum() / mask.sum()

    return intra + inter + ce


def kernel(embedding, prediction, class_label, neighbor):
    embedding = np.ascontiguousarray(np.asarray(embedding), dtype=np.float32)
    prediction = np.ascontiguousarray(np.asarray(prediction), dtype=np.float32)
    class_label = np.ascontiguousarray(np.asarray(class_label), dtype=np.int32)
    neighbor = np.asarray(neighbor)

    nc = _get_nc()
    in_maps = []
    for core in range(NCORES):
        sl = slice(core * BLOC, (core + 1) * BLOC)
        in_maps.append(
            {
                "emb": embedding[sl].reshape(BLOC, E, P),
                "pred": prediction[sl].reshape(BLOC, L, P),
                "lab": class_label[sl].reshape(BLOC, P),
            }
        )
    out = run_bass_kernel_spmd(nc, in_maps, core_ids=list(range(NCORES)))

    total = 0.0
    for core in range(NCORES):
        for i in range(BLOC):
            b = core * BLOC + i
            total += _host_epilogue(out.results[core]["res"][i], neighbor[b])
    return np.float32(total)

